# revision 12
# baseline (speedup 1.0000x reference)
"""Trainium2 Bass kernel for DiagramNet retrieval-knn.

Computation (per batch example b):
  sim[m,n]   = <dia[b,n,:], dd[b,m,n,:]> / max(|dia[b,n]| * |dd[b,m,n]|, EPS)
  avg[m]     = sum_n sim[m,n] / count_n(dd[b,m,n] not all-zero)   (NEG_BIG if count==0)
  v, ix      = max_m avg, argmax_m avg
  out[b]     = dd[b,ix] if v > 0.5 else dia[b]

Sharding: data-parallel over batch B=32 across 8 cores (4 examples/core).

Layout strategy (per example, per core):
  Flatten (m,n) -> 8192 rows of D=256. Each SBUF partition p of a chunk
  holds R consecutive rows (R*1KB contiguous DRAM per partition -> good DMA).
  chunk c, partition p, slice j  <->  flat = c*128*R + p*R + j,
  m = flat // 64, n = flat % 64. So m = c*2R + p//G, n = R*(p%G) + j  (G = 64/R).

  num (dot with dia) via VectorE tensor_tensor_reduce (fused mul+row-reduce),
  sum-of-squares via ScalarE activation(Square, accum_out) - the two heavy
  passes run on different engines in parallel while DMA streams chunks.

  n-sums via PE matmuls with per-group indicator lhsT -> [1, 128] PSUM rows
  in true m-order, then per-partition max/max_index for v/argmax, and a
  data-dependent DMA (dynamic slice by register) for the final gather.
"""

import os
import sys

for _p in ("/opt/trn_rl_repo", "/root/.axon_site/_ro/trn_rl_repo"):
    if os.path.isdir(_p) and _p not in sys.path:
        sys.path.insert(0, _p)

import numpy as np

import bass_rust
import concourse.bass as bass
import concourse.mybir as mybir
import concourse.tile as tile
from concourse.bass_utils import run_bass_kernel_spmd
from concourse.vector_clock import ScopedClock

# --- workaround: this toolchain's walrus accepts at most 1 sync-wait per
# instruction (2 for EventSemaphore), but Tile sometimes attaches more
# (notably the kernel-tail Drain, and occasionally compute ops). Post-pass:
# move excess waits onto single-wait NoOps inserted just before the owner.
def _split_excess_waits(nc: bass.Bass) -> None:
    n_split = 0
    for f in nc.m.functions:
        for bb in f.blocks:
            new_insts = []
            changed = False
            for inst in list(bb.instructions):
                si = inst.sync_info
                waits = list(si.on_wait) if si is not None and si.on_wait else []
                cap = 2 if isinstance(inst, mybir.InstEventSemaphore) else 1
                if len(waits) > cap:
                    changed = True
                    for w in waits[:-cap]:
                        nop = mybir.InstNoOp(
                            name=f"waitsplit-{n_split}", ins=[], outs=[]
                        )
                        n_split += 1
                        nop.engine = inst.engine
                        nop.sync_info = mybir.SyncInfo(on_wait=[w], on_update=[])
                        new_insts.append(nop)
                    si.on_wait = waits[-cap:]
                new_insts.append(inst)
            if changed:
                bb.instructions = new_insts

F32 = mybir.dt.float32
U32 = mybir.dt.uint32
ALU = mybir.AluOpType
ACTF = mybir.ActivationFunctionType
AX = mybir.AxisListType

B, M, N, D = 32, 128, 64, 256
NCORES = 8
BLOC = B // NCORES  # 4 examples per core
EPS = 1e-8
NEG_BIG = -9e15

R = 4            # flat (m,n)-rows per partition per chunk (contiguity = R KB)
G = N // R       # partitions per m-group
MPC = 2 * R      # m's per chunk
C = M // MPC     # chunks per example


def build_nc(bloc: int = BLOC, split_waits: bool = True) -> bass.Bass:
    nc = bass.Bass()
    dia = nc.dram_tensor("dia", [bloc, N, D], F32, kind="ExternalInput")
    dd = nc.dram_tensor("dd", [bloc, M, N, D], F32, kind="ExternalInput")
    out = nc.dram_tensor("out", [bloc, N, D], F32, kind="ExternalOutput")

    from contextlib import ExitStack

    with tile.TileContext(nc) as tc, ExitStack() as ctx:
        const_pool = ctx.enter_context(tc.tile_pool(name="const", bufs=1))
        ex_pool = ctx.enter_context(tc.tile_pool(name="ex", bufs=2))
        dd_pool = ctx.enter_context(tc.tile_pool(name="ddp", bufs=4))
        scr_pool = ctx.enter_context(tc.tile_pool(name="scr", bufs=3))
        small_pool = ctx.enter_context(tc.tile_pool(name="small", bufs=2))
        psum_pool = ctx.enter_context(tc.tile_pool(name="psum", bufs=2, space="PSUM"))

        # Indicator matrix for per-m-group partition sums: ind[p, g] = (p//G == g).
        # Built as (0 <= p - G*g < G) from an affine iota t[p, g] = p - G*g.
        it = const_pool.tile([128, MPC], mybir.dt.int32)
        nc.gpsimd.iota(it, pattern=[[-G, MPC]], base=0, channel_multiplier=1)
        ind_ge = const_pool.tile([128, MPC], F32)
        nc.vector.tensor_scalar(ind_ge, it, 0, scalar2=None, op0=ALU.is_ge)
        ind_lt = const_pool.tile([128, MPC], F32)
        nc.vector.tensor_scalar(ind_lt, it, G, scalar2=None, op0=ALU.is_lt)
        ind = const_pool.tile([128, MPC], F32)
        nc.vector.tensor_mul(ind, ind_ge, ind_lt)

        # ones row for PE partition-broadcast ([1,1] scalar -> [128,1])
        ones1 = const_pool.tile([1, 128], F32)
        nc.vector.memset(ones1, 1.0)
        # per-example partition iota (float): p + b*M*N, for gather indices
        iota_f = const_pool.tile([128, bloc], F32)
        iota_i = const_pool.tile([128, bloc], mybir.dt.int32)
        nc.gpsimd.iota(iota_i, pattern=[[M * N, bloc]], base=0, channel_multiplier=1)
        nc.vector.tensor_copy(iota_f, iota_i)

        dd_rows = dd.rearrange("b m n d -> (b m n) d")

        for b in range(bloc):
            # ---- dia setup: partition p holds dia rows [R*(p%G), R*(p%G)+R) ----
            dia2w = ex_pool.tile([128, R * D], F32)
            dia_g = dia[b].rearrange("(g r) d -> g (r d)", g=G)
            dia_bc = bass.AP(
                tensor=dia_g.tensor,
                offset=dia_g.offset,
                ap=[[0, 128 // G]] + list(dia_g.ap),
            )
            nc.sync.dma_start(out=dia2w, in_=dia_bc)

            dn_sq = ex_pool.tile([128, R], F32)
            for j in range(R):
                scr_a = scr_pool.tile([128, D], F32, tag="scr_a")
                nc.scalar.activation(
                    out=scr_a,
                    in_=dia2w[:, j * D : (j + 1) * D],
                    func=ACTF.Square,
                    accum_out=dn_sq[:, j : j + 1],
                )
            dianorm_w = ex_pool.tile([128, R], F32)
            nc.scalar.sqrt(dianorm_w, dn_sq)

            # ---- main streaming loop: num and ssq for all (m, n) ----
            num_all = ex_pool.tile([128, N], F32)
            ssq_all = ex_pool.tile([128, N], F32)
            dd_flat = dd[b].rearrange("m n d -> (m n) d")
            for c in range(C):
                dd_t = dd_pool.tile([128, R, D], F32)
                src = dd_flat[c * 128 * R : (c + 1) * 128 * R].rearrange(
                    "(p r) d -> p r d", r=R
                )
                nc.sync.dma_start(out=dd_t, in_=src)
                for j in range(R):
                    k = c * R + j
                    scr_v = scr_pool.tile([128, D], F32, tag="scr_v")
                    scr_s = scr_pool.tile([128, D], F32, tag="scr_s")
                    nc.vector.scalar_tensor_tensor(
                        out=scr_v,
                        in0=dd_t[:, j, :],
                        scalar=1.0,
                        in1=dia2w[:, j * D : (j + 1) * D],
                        op0=ALU.mult,
                        op1=ALU.mult,
                        accum_out=num_all[:, k : k + 1],
                    )
                    nc.scalar.activation(
                        out=scr_s,
                        in_=dd_t[:, j, :],
                        func=ACTF.Square,
                        accum_out=ssq_all[:, k : k + 1],
                    )

            # ---- per-(m,n) similarity ----
            ddnorm = ex_pool.tile([128, N], F32)
            nc.scalar.sqrt(ddnorm, ssq_all)
            denom = ex_pool.tile([128, N], F32)
            dn_bc = bass.AP(
                tensor=dianorm_w.tensor,
                offset=dianorm_w.offset,
                ap=[dianorm_w.ap[0], [0, C], dianorm_w.ap[1]],
            )
            nc.vector.tensor_mul(
                denom.rearrange("p (c r) -> p c r", r=R),
                ddnorm.rearrange("p (c r) -> p c r", r=R),
                dn_bc,
            )
            nc.vector.tensor_scalar_max(denom, denom, EPS)
            rden = ex_pool.tile([128, N], F32)
            nc.vector.reciprocal(rden, denom)
            stage = ex_pool.tile([128, 2 * N], F32)
            nc.vector.tensor_mul(stage[:, 0:N], num_all, rden)  # sim
            nc.vector.tensor_scalar(
                stage[:, N : 2 * N], ssq_all, 0.0, scalar2=None, op0=ALU.is_gt
            )  # 1.0 where dd row non-zero

            # ---- n-sums into true-m-order [1, 128] rows ----
            simsum = small_pool.tile([1, M], F32)
            ddnum = small_pool.tile([1, M], F32)
            for g in range(MPC):
                pg = psum_pool.tile([1, 2 * N], F32)
                nc.tensor.matmul(
                    pg, lhsT=ind[:, g : g + 1], rhs=stage, start=True, stop=True
                )
                sview = simsum.rearrange("p (c s) -> p c s", s=MPC)[:, :, g]
                nc.vector.reduce_sum(
                    out=sview,
                    in_=pg[:, 0:N].rearrange("p (c r) -> p c r", r=R),
                    axis=AX.X,
                )
                dview = ddnum.rearrange("p (c s) -> p c s", s=MPC)[:, :, g]
                nc.vector.reduce_sum(
                    out=dview,
                    in_=pg[:, N : 2 * N].rearrange("p (c r) -> p c r", r=R),
                    axis=AX.X,
                )

            # ---- avg = simsum / where(ddnum==0, NEG_BIG, ddnum) ----
            is0 = small_pool.tile([1, M], F32)
            nc.vector.tensor_scalar(is0, ddnum, 0.0, scalar2=None, op0=ALU.is_equal)
            ddn2 = small_pool.tile([1, M], F32)
            nc.vector.scalar_tensor_tensor(
                out=ddn2, in0=is0, scalar=NEG_BIG, in1=ddnum,
                op0=ALU.mult, op1=ALU.add,
            )
            rddn = small_pool.tile([1, M], F32)
            nc.vector.reciprocal(rddn, ddn2)
            avg = small_pool.tile([1, M], F32)
            nc.vector.tensor_mul(avg, simsum, rddn)

            # ---- v / argmax ----
            max8 = small_pool.tile([1, 8], F32)
            idx8 = small_pool.tile([1, 8], U32)
            nc.vector.max(out=max8, in_=avg)
            nc.vector.max_index(out=idx8, in_max=max8, in_values=avg)
            idxf = small_pool.tile([1, 1], F32)
            nc.vector.tensor_copy(idxf, idx8[:, 0:1])  # u32 -> f32
            flag = small_pool.tile([1, 1], F32)
            nc.vector.tensor_scalar(
                flag, max8[:, 0:1], 0.5, scalar2=None, op0=ALU.is_gt
            )

            # broadcast m* and the select flag to all partitions via PE
            ps_m = psum_pool.tile([128, 1], F32, tag="ps_bcast")
            nc.tensor.matmul(ps_m, lhsT=ones1, rhs=idxf, start=True, stop=True)
            ps_s = psum_pool.tile([128, 1], F32, tag="ps_bcast")
            nc.tensor.matmul(ps_s, lhsT=ones1, rhs=flag, start=True, stop=True)
            s_sb = small_pool.tile([128, 1], F32)
            nc.vector.tensor_copy(s_sb, ps_s)

            # gather row indices: idx[p] = b*M*N + m* * N + p   (p = n)
            idxg = small_pool.tile([128, 1], U32)
            nc.vector.scalar_tensor_tensor(
                out=idxg, in0=ps_m, scalar=float(N), in1=iota_f[:, b : b + 1],
                op0=ALU.mult, op1=ALU.add,
            )
            closest = ex_pool.tile([N, D], F32)
            nc.gpsimd.indirect_dma_start(
                out=closest[:],
                out_offset=None,
                in_=dd_rows[:],
                in_offset=bass.IndirectOffsetOnAxis(ap=idxg[0:N, :], axis=0),
            )

            # blend: out = dia + s * (closest - dia)
            dia_nat = ex_pool.tile([N, D], F32)
            nc.sync.dma_start(out=dia_nat, in_=dia[b])
            diff = ex_pool.tile([N, D], F32)
            nc.vector.tensor_sub(diff, closest, dia_nat)
            outt = ex_pool.tile([N, D], F32)
            nc.vector.scalar_tensor_tensor(
                out=outt, in0=diff, scalar=s_sb[0:N, :], in1=dia_nat,
                op0=ALU.mult, op1=ALU.add,
            )
            nc.sync.dma_start(out=out[b], in_=outt)

    if split_waits:
        _split_excess_waits(nc)
    return nc


_NC_CACHE: dict[int, bass.Bass] = {}


def _get_nc(bloc: int = BLOC) -> bass.Bass:
    nc = _NC_CACHE.get(bloc)
    if nc is None:
        nc = build_nc(bloc)
        _NC_CACHE[bloc] = nc
    return nc


LAST_RESULTS = None  # BassKernelResults of the most recent run (for profiling)


def kernel(dia_node_feat: np.ndarray, dd_node_feat: np.ndarray) -> np.ndarray:
    dia = np.ascontiguousarray(np.asarray(dia_node_feat, dtype=np.float32))
    dd = np.ascontiguousarray(np.asarray(dd_node_feat, dtype=np.float32))
    assert dia.shape == (B, N, D) and dd.shape == (B, M, N, D)

    nc = _get_nc()
    in_maps = [
        {
            "dia": dia[i * BLOC : (i + 1) * BLOC],
            "dd": dd[i * BLOC : (i + 1) * BLOC],
        }
        for i in range(NCORES)
    ]
    trace = os.environ.get("BASS_KERNEL_TRACE", "0") == "1"
    kwargs = {}
    if trace:
        kwargs["trace"] = True
        kwargs["trace_cores"] = list(range(NCORES))
    res = run_bass_kernel_spmd(nc, in_maps, core_ids=list(range(NCORES)), **kwargs)
    global LAST_RESULTS
    LAST_RESULTS = res
    return np.concatenate([r["out"] for r in res.results], axis=0)


# revision 14
# speedup vs baseline: 38440.4832x; 38440.4832x over previous
"""Trainium2 Bass kernel for DiagramNet retrieval-knn.

Computation (per batch example b):
  sim[m,n]   = <dia[b,n,:], dd[b,m,n,:]> / max(|dia[b,n]| * |dd[b,m,n]|, EPS)
  avg[m]     = sum_n sim[m,n] / count_n(dd[b,m,n] not all-zero)   (NEG_BIG if count==0)
  v, ix      = max_m avg, argmax_m avg
  out[b]     = dd[b,ix] if v > 0.5 else dia[b]

Sharding: data-parallel over batch B=32 across 8 cores (4 examples/core).

Layout strategy (per example, per core):
  Flatten (m,n) -> 8192 rows of D=256. Each SBUF partition p of a chunk
  holds R consecutive rows (R*1KB contiguous DRAM per partition -> good DMA).
  chunk c, partition p, slice j  <->  flat = c*128*R + p*R + j,
  m = flat // 64, n = flat % 64. So m = c*2R + p//G, n = R*(p%G) + j  (G = 64/R).

  num (dot with dia) via VectorE tensor_tensor_reduce (fused mul+row-reduce),
  sum-of-squares via ScalarE activation(Square, accum_out) - the two heavy
  passes run on different engines in parallel while DMA streams chunks.

  n-sums via PE matmuls with per-group indicator lhsT -> [1, 128] PSUM rows
  in true m-order, then per-partition max/max_index for v/argmax, and a
  data-dependent DMA (dynamic slice by register) for the final gather.
"""

import os
import sys

for _p in ("/opt/trn_rl_repo", "/root/.axon_site/_ro/trn_rl_repo"):
    if os.path.isdir(_p) and _p not in sys.path:
        sys.path.insert(0, _p)

import numpy as np

import bass_rust
import concourse.bass as bass
import concourse.mybir as mybir
import concourse.tile as tile
from concourse.bass_utils import run_bass_kernel_spmd
from concourse.vector_clock import ScopedClock

# --- workaround: this toolchain's walrus accepts at most 1 sync-wait per
# instruction (2 for EventSemaphore), but Tile sometimes attaches more
# (notably the kernel-tail Drain, and occasionally compute ops). Post-pass:
# move excess waits onto single-wait NoOps inserted just before the owner.
def _split_excess_waits(nc: bass.Bass) -> None:
    n_split = 0
    for f in nc.m.functions:
        for bb in f.blocks:
            new_insts = []
            changed = False
            for inst in list(bb.instructions):
                si = inst.sync_info
                waits = list(si.on_wait) if si is not None and si.on_wait else []
                cap = 2 if isinstance(inst, mybir.InstEventSemaphore) else 1
                if len(waits) > cap:
                    changed = True
                    for w in waits[:-cap]:
                        nop = mybir.InstNoOp(
                            name=f"waitsplit-{n_split}", ins=[], outs=[]
                        )
                        n_split += 1
                        nop.engine = inst.engine
                        nop.sync_info = mybir.SyncInfo(on_wait=[w], on_update=[])
                        new_insts.append(nop)
                    si.on_wait = waits[-cap:]
                new_insts.append(inst)
            if changed:
                bb.instructions = new_insts

F32 = mybir.dt.float32
U32 = mybir.dt.uint32
ALU = mybir.AluOpType
ACTF = mybir.ActivationFunctionType
AX = mybir.AxisListType

B, M, N, D = 32, 128, 64, 256
NCORES = 8
BLOC = B // NCORES  # 4 examples per core
EPS = 1e-8
NEG_BIG = -9e15

R = 4            # flat (m,n)-rows per partition per chunk (contiguity = R KB)
G = N // R       # partitions per m-group
MPC = 2 * R      # m's per chunk
C = M // MPC     # chunks per example


def build_nc(bloc: int = BLOC, split_waits: bool = True) -> bass.Bass:
    nc = bass.Bass()
    dia = nc.dram_tensor("dia", [bloc, N, D], F32, kind="ExternalInput")
    dd = nc.dram_tensor("dd", [bloc, M, N, D], F32, kind="ExternalInput")
    out = nc.dram_tensor("out", [bloc, N, D], F32, kind="ExternalOutput")

    from contextlib import ExitStack

    with tile.TileContext(nc) as tc, ExitStack() as ctx:
        const_pool = ctx.enter_context(tc.tile_pool(name="const", bufs=1))
        ex_pool = ctx.enter_context(tc.tile_pool(name="ex", bufs=2))
        dd_pool = ctx.enter_context(tc.tile_pool(name="ddp", bufs=4))
        scr_pool = ctx.enter_context(tc.tile_pool(name="scr", bufs=3))
        small_pool = ctx.enter_context(tc.tile_pool(name="small", bufs=2))
        psum_pool = ctx.enter_context(tc.tile_pool(name="psum", bufs=2, space="PSUM"))

        # Indicator matrix for per-m-group partition sums: ind[p, g] = (p//G == g).
        # Built as (0 <= p - G*g < G) from an affine iota t[p, g] = p - G*g.
        it = const_pool.tile([128, MPC], mybir.dt.int32)
        nc.gpsimd.iota(it, pattern=[[-G, MPC]], base=0, channel_multiplier=1)
        ind_ge = const_pool.tile([128, MPC], F32)
        nc.vector.tensor_scalar(ind_ge, it, 0, scalar2=None, op0=ALU.is_ge)
        ind_lt = const_pool.tile([128, MPC], F32)
        nc.vector.tensor_scalar(ind_lt, it, G, scalar2=None, op0=ALU.is_lt)
        ind = const_pool.tile([128, MPC], F32)
        nc.vector.tensor_mul(ind, ind_ge, ind_lt)

        # ones row for PE partition-broadcast ([1,1] scalar -> [128,1])
        ones1 = const_pool.tile([1, 128], F32)
        nc.vector.memset(ones1, 1.0)
        # per-example partition iota (float): p + b*M*N, for gather indices
        iota_f = const_pool.tile([128, bloc], F32)
        iota_i = const_pool.tile([128, bloc], mybir.dt.int32)
        nc.gpsimd.iota(iota_i, pattern=[[M * N, bloc]], base=0, channel_multiplier=1)
        nc.vector.tensor_copy(iota_f, iota_i)

        dd_rows = dd.rearrange("b m n d -> (b m n) d")

        for b in range(bloc):
            # ---- dia setup: partition p holds dia rows [R*(p%G), R*(p%G)+R) ----
            dia2w = ex_pool.tile([128, R * D], F32)
            dia_g = dia[b].rearrange("(g r) d -> g (r d)", g=G)
            dia_bc = bass.AP(
                tensor=dia_g.tensor,
                offset=dia_g.offset,
                ap=[[0, 128 // G]] + list(dia_g.ap),
            )
            nc.sync.dma_start(out=dia2w, in_=dia_bc)

            dn_sq = ex_pool.tile([128, R], F32)
            for j in range(R):
                scr_a = scr_pool.tile([128, D], F32, tag="scr_a")
                nc.scalar.activation(
                    out=scr_a,
                    in_=dia2w[:, j * D : (j + 1) * D],
                    func=ACTF.Square,
                    accum_out=dn_sq[:, j : j + 1],
                )
            dianorm_w = ex_pool.tile([128, R], F32)
            nc.scalar.sqrt(dianorm_w, dn_sq)

            # ---- main streaming loop: num and ssq for all (m, n) ----
            # Work split across engines (cost-model balanced):
            #   num (dd . dia): DVE STT w/ accum (3/4) + GpSimd STT (1/4)
            #   ssq (dd . dd):  ACT Square w/ accum (1/2) + GpSimd STT (1/2)
            # DMAs round-robin over the 2 HWDGE queues (SP and ACT sequencers).
            num_all = ex_pool.tile([128, N], F32)
            ssq_all = ex_pool.tile([128, N], F32)
            dd_flat = dd[b].rearrange("m n d -> (m n) d")
            dma_engines = [nc.sync, nc.scalar]
            for c in range(C):
                dd_t = dd_pool.tile([128, R, D], F32)
                src = dd_flat[c * 128 * R : (c + 1) * 128 * R].rearrange(
                    "(p r) d -> p r d", r=R
                )
                dma_engines[(b * C + c) % len(dma_engines)].dma_start(
                    out=dd_t, in_=src
                )
                for j in range(R):
                    k = c * R + j
                    scr_v = scr_pool.tile([128, D], F32, tag="scr_v")
                    scr_s = scr_pool.tile([128, D], F32, tag="scr_s")
                    num_eng = nc.gpsimd if k % 4 == 1 else nc.vector
                    num_eng.scalar_tensor_tensor(
                        out=scr_v,
                        in0=dd_t[:, j, :],
                        scalar=1.0,
                        in1=dia2w[:, j * D : (j + 1) * D],
                        op0=ALU.mult,
                        op1=ALU.mult,
                        accum_out=num_all[:, k : k + 1],
                    )
                    if k % 2 == 0:
                        nc.scalar.activation(
                            out=scr_s,
                            in_=dd_t[:, j, :],
                            func=ACTF.Square,
                            accum_out=ssq_all[:, k : k + 1],
                        )
                    else:
                        nc.gpsimd.scalar_tensor_tensor(
                            out=scr_s,
                            in0=dd_t[:, j, :],
                            scalar=1.0,
                            in1=dd_t[:, j, :],
                            op0=ALU.mult,
                            op1=ALU.mult,
                            accum_out=ssq_all[:, k : k + 1],
                        )

            # ---- per-(m,n) similarity ----
            ddnorm = ex_pool.tile([128, N], F32)
            nc.scalar.sqrt(ddnorm, ssq_all)
            denom = ex_pool.tile([128, N], F32)
            dn_bc = bass.AP(
                tensor=dianorm_w.tensor,
                offset=dianorm_w.offset,
                ap=[dianorm_w.ap[0], [0, C], dianorm_w.ap[1]],
            )
            nc.vector.tensor_mul(
                denom.rearrange("p (c r) -> p c r", r=R),
                ddnorm.rearrange("p (c r) -> p c r", r=R),
                dn_bc,
            )
            nc.vector.tensor_scalar_max(denom, denom, EPS)
            rden = ex_pool.tile([128, N], F32)
            nc.vector.reciprocal(rden, denom)
            stage = ex_pool.tile([128, 2 * N], F32)
            nc.vector.tensor_mul(stage[:, 0:N], num_all, rden)  # sim
            nc.vector.tensor_scalar(
                stage[:, N : 2 * N], ssq_all, 0.0, scalar2=None, op0=ALU.is_gt
            )  # 1.0 where dd row non-zero

            # ---- n-sums into true-m-order [1, 128] rows ----
            simsum = small_pool.tile([1, M], F32)
            ddnum = small_pool.tile([1, M], F32)
            for g in range(MPC):
                pg = psum_pool.tile([1, 2 * N], F32)
                nc.tensor.matmul(
                    pg, lhsT=ind[:, g : g + 1], rhs=stage, start=True, stop=True
                )
                sview = simsum.rearrange("p (c s) -> p c s", s=MPC)[:, :, g]
                nc.vector.reduce_sum(
                    out=sview,
                    in_=pg[:, 0:N].rearrange("p (c r) -> p c r", r=R),
                    axis=AX.X,
                )
                dview = ddnum.rearrange("p (c s) -> p c s", s=MPC)[:, :, g]
                nc.vector.reduce_sum(
                    out=dview,
                    in_=pg[:, N : 2 * N].rearrange("p (c r) -> p c r", r=R),
                    axis=AX.X,
                )

            # ---- avg = simsum / where(ddnum==0, NEG_BIG, ddnum) ----
            is0 = small_pool.tile([1, M], F32)
            nc.vector.tensor_scalar(is0, ddnum, 0.0, scalar2=None, op0=ALU.is_equal)
            ddn2 = small_pool.tile([1, M], F32)
            nc.vector.scalar_tensor_tensor(
                out=ddn2, in0=is0, scalar=NEG_BIG, in1=ddnum,
                op0=ALU.mult, op1=ALU.add,
            )
            rddn = small_pool.tile([1, M], F32)
            nc.vector.reciprocal(rddn, ddn2)
            avg = small_pool.tile([1, M], F32)
            nc.vector.tensor_mul(avg, simsum, rddn)

            # ---- v / argmax ----
            max8 = small_pool.tile([1, 8], F32)
            idx8 = small_pool.tile([1, 8], U32)
            nc.vector.max(out=max8, in_=avg)
            nc.vector.max_index(out=idx8, in_max=max8, in_values=avg)
            idxf = small_pool.tile([1, 1], F32)
            nc.vector.tensor_copy(idxf, idx8[:, 0:1])  # u32 -> f32
            flag = small_pool.tile([1, 1], F32)
            nc.vector.tensor_scalar(
                flag, max8[:, 0:1], 0.5, scalar2=None, op0=ALU.is_gt
            )

            # broadcast m* and the select flag to all partitions via PE
            ps_m = psum_pool.tile([128, 1], F32, tag="ps_bcast")
            nc.tensor.matmul(ps_m, lhsT=ones1, rhs=idxf, start=True, stop=True)
            ps_s = psum_pool.tile([128, 1], F32, tag="ps_bcast")
            nc.tensor.matmul(ps_s, lhsT=ones1, rhs=flag, start=True, stop=True)
            s_sb = small_pool.tile([128, 1], F32)
            nc.vector.tensor_copy(s_sb, ps_s)

            # gather row indices: idx[p] = b*M*N + m* * N + p   (p = n)
            idxg = small_pool.tile([128, 1], U32)
            nc.vector.scalar_tensor_tensor(
                out=idxg, in0=ps_m, scalar=float(N), in1=iota_f[:, b : b + 1],
                op0=ALU.mult, op1=ALU.add,
            )
            closest = ex_pool.tile([N, D], F32)
            nc.gpsimd.indirect_dma_start(
                out=closest[:],
                out_offset=None,
                in_=dd_rows[:],
                in_offset=bass.IndirectOffsetOnAxis(ap=idxg[0:N, :], axis=0),
            )

            # blend: out = dia + s * (closest - dia)
            dia_nat = ex_pool.tile([N, D], F32)
            nc.sync.dma_start(out=dia_nat, in_=dia[b])
            diff = ex_pool.tile([N, D], F32)
            nc.vector.tensor_sub(diff, closest, dia_nat)
            outt = ex_pool.tile([N, D], F32)
            nc.vector.scalar_tensor_tensor(
                out=outt, in0=diff, scalar=s_sb[0:N, :], in1=dia_nat,
                op0=ALU.mult, op1=ALU.add,
            )
            nc.sync.dma_start(out=out[b], in_=outt)

    if split_waits:
        _split_excess_waits(nc)
    return nc


_NC_CACHE: dict[int, bass.Bass] = {}


def _get_nc(bloc: int = BLOC) -> bass.Bass:
    nc = _NC_CACHE.get(bloc)
    if nc is None:
        nc = build_nc(bloc)
        _NC_CACHE[bloc] = nc
    return nc


LAST_RESULTS = None  # BassKernelResults of the most recent run (for profiling)


def kernel(dia_node_feat: np.ndarray, dd_node_feat: np.ndarray) -> np.ndarray:
    dia = np.ascontiguousarray(np.asarray(dia_node_feat, dtype=np.float32))
    dd = np.ascontiguousarray(np.asarray(dd_node_feat, dtype=np.float32))
    assert dia.shape == (B, N, D) and dd.shape == (B, M, N, D)

    nc = _get_nc()
    in_maps = [
        {
            "dia": dia[i * BLOC : (i + 1) * BLOC],
            "dd": dd[i * BLOC : (i + 1) * BLOC],
        }
        for i in range(NCORES)
    ]
    trace = os.environ.get("BASS_KERNEL_TRACE", "0") == "1"
    kwargs = {}
    if trace:
        kwargs["trace"] = True
        kwargs["trace_cores"] = list(range(NCORES))
    res = run_bass_kernel_spmd(nc, in_maps, core_ids=list(range(NCORES)), **kwargs)
    global LAST_RESULTS
    LAST_RESULTS = res
    return np.concatenate([r["out"] for r in res.results], axis=0)
